# revision 20
# baseline (speedup 1.0000x reference)
"""Grouped GEMM (MoE routing) Trainium2 kernel — token-streaming fp8 design.

Expert-parallel across 8 NeuronCores with size-sorted slot assignment
(slot s on core c holds the expert of size-rank 8s+c; per-slot capacity
cap_s = roundup4(max count in rank group)).

Key design vs the bf16 token-stationary baseline:
- Weights are quantized to float8e3 (E3M4, 4 mantissa bits) on host with
  a global power-of-2 scale folded into x (y = (x/s) @ (w*s) exactly), so
  weight HBM traffic halves: 68 MB -> 34 MB per core.  x stays bf16 as
  the PE moving operand (mixed-dtype matmul), out written bf16.
- Token-streaming orientation: stationary = w tile [128k x 128n] fp8,
  moving = x^T [128k x cap] bf16, psum [128n x cap].  PE cost scales with
  actual token count instead of ceil(count/128)*128.
- PSUM bank-group rotation: each slot runs 4 n-phases (0,4)(4,8)(8,12)
  (12,13) on alternating bank groups 0-3 / 4-7, so a phase's first matmul
  reuses banks whose psum->sbuf copies completed a full phase earlier —
  no copy-latency bubble at phase/slot boundaries (the bubbles triggered
  the HAM clock governor to halve the PE clock in the small-cap tail).
- Per-phase output DMA keeps the kernel-exit DMA tiny (1-bank phase).
"""
import ml_dtypes
import numpy as np

import concourse.mybir as mybir
import concourse.tile as tile
from concourse import bacc
from concourse.bass_utils import run_bass_kernel_spmd

G, T, DIN, DOUT = 64, 8192, 2560, 1664
NCORES = 8
EPC = G // NCORES   # expert slots per core
KC = DIN // 128     # 20 contraction chunks
NN = DOUT // 128    # 13 output-row chunks

_cache = {}


def _build(caps):
    caps = [int(c) for c in caps if c > 0]
    offs = np.concatenate([[0], np.cumsum(caps)]).astype(int)
    S = int(offs[-1])
    nc = bacc.Bacc(trn_type="TRN2", debug=False)
    f8 = mybir.dt.float8e3
    bf16 = mybir.dt.bfloat16
    f32 = mybir.dt.float32

    # partition-major layouts: every DMA below is a [128, N] slice whose
    # per-partition bytes are contiguous in HBM (large descriptors)
    w8 = nc.dram_tensor("w8", [EPC, 128, KC * DOUT], f8,
                        kind="ExternalInput").ap()
    xt = nc.dram_tensor("xt", [128, KC * S], bf16, kind="ExternalInput").ap()
    out = nc.dram_tensor("out", [128, NN * S], bf16, kind="ExternalOutput").ap()

    WB = 4   # k-chunks per w DMA
    NWCH = KC // WB  # 5 w chunks per slot
    with tile.TileContext(nc) as tc:
        with (
            tc.tile_pool(name="wp", bufs=3) as w_pool,
            tc.tile_pool(name="xp", bufs=1) as x_pool,
            tc.tile_pool(name="op", bufs=2) as o_pool,
            tc.tile_pool(name="ps", bufs=1, space="PSUM") as ps_pool,
        ):
            # PE warm-up: ~6us of dummy matmuls on a zeroed tile so the HAM
            # clock gate reaches 2.4 GHz before the first real matmul, and the
            # PE is busy while the first DMAs land.
            warm_l = x_pool.tile([128, 128], bf16, tag="wl", name="warm_l")
            warm_r = x_pool.tile([128, 512], bf16, tag="wr", name="warm_r")
            nc.vector.memset(warm_l[:], 0)
            nc.vector.memset(warm_r[:], 0)
            pswarm = ps_pool.tile([128, 512], f32, tag="psw", name="pswarm")
            for i in range(30):
                nc.tensor.matmul(pswarm[:], warm_l[:], warm_r[:],
                                 start=True, stop=True)
            psums = {}
            for j in range(7):  # one open accumulation region per bank
                psums[j] = ps_pool.tile([128, 512], f32, tag=f"ps{j}",
                                        name=f"psum{j}")
            psums[7] = pswarm  # warmup bank doubles as the 8th region

            for s, cap in enumerate(caps):
                off = int(offs[s])
                # per-slot x^T tile (1.3MB at cap 256): prefetched like the
                # weights; slots 0/1 ride the fast HWDGE rings so slot 0
                # starts right as the warmup ends, later slots go SWDGE
                xs = x_pool.tile([128, KC * cap], bf16, tag="xs",
                                 name=f"xs{s}", bufs=3)
                # all x^T traffic rides the gpsimd (SWDGE) ring so the two
                # HWDGE rings carry nothing but the weight stream; slot 0's
                # x is split so its head chunks land first
                if s == 0:
                    h = (KC // 2) * cap
                    nc.gpsimd.dma_start(xs[:, :h], xt[:, KC * off:KC * off + h])
                    nc.gpsimd.dma_start(
                        xs[:, h:], xt[:, KC * off + h:KC * (off + cap)]
                    )
                else:
                    nc.gpsimd.dma_start(xs[:], xt[:, KC * off:KC * (off + cap)])
                # 852KB weight chunks strictly alternate the two HWDGE rings
                # (global parity, so consecutive chunks always stream in
                # parallel); the gpsimd ring takes one mid chunk of the tail
                # slots, whose per-slot weight demand rate exceeds what two
                # rings deliver.  Fine WB=4 chunks with 3-4 deep rings free
                # buffers smoothly — coarser chunks cluster the refills into
                # waves that stall the stream.
                wch = {}
                for j in range(NWCH):
                    wj = w_pool.tile([128, WB * DOUT], f8, tag=f"w{j}",
                                     name=f"w{s}_{j}", bufs=4)
                    if s >= 5 and j == 2:
                        eng = nc.gpsimd
                    else:
                        eng = nc.sync if (s * NWCH + j) % 2 == 0 else nc.scalar
                    eng.dma_start(
                        wj[:], w8[s, :, j * WB * DOUT:(j + 1) * WB * DOUT]
                    )
                    for k in range(j * WB, (j + 1) * WB):
                        wch[k] = (wj, (k - j * WB) * DOUT)
                o_sb = o_pool.tile([128, NN * cap], bf16, tag="o", name=f"o{s}")
                assert cap <= 256
                # k-outer within each n-phase.  Wide slots use two wide
                # phases — their weight chunks are consumed over most of the
                # slot, matching the DMA stream rate.  Narrow slots use four
                # rotated 4-bank phases (groups 0-3 / 4-7 alternating), so a
                # phase's first matmul reuses banks whose psum->sbuf copies
                # completed a full phase earlier — no copy-latency bubble at
                # the boundaries (those bubbles triggered the HAM clock
                # governor to halve the PE clock).  The last slot splits its
                # final phase so the kernel-exit output DMA is one bank.
                if cap <= 96:
                    phases = ((0, 4, 0), (4, 8, 4), (8, 12, 0), (12, NN, 4))
                elif s == len(caps) - 1:
                    phases = ((0, 8, 0), (8, 12, 0), (12, NN, 4))
                else:
                    phases = ((0, 8, 0), (8, NN, 0))
                for n0, n1, bank0 in phases:
                    for k in range(KC):
                        wk, kb = wch[k]
                        for n in range(n0, n1):
                            ps = psums[bank0 + n - n0][:, :cap]
                            nc.tensor.matmul(
                                ps,
                                wk[:, kb + n * 128:kb + (n + 1) * 128],
                                xs[:, k * cap:(k + 1) * cap],
                                start=(k == 0),
                                stop=(k == KC - 1),
                            )
                    for n in range(n0, n1):
                        nc.vector.tensor_copy(
                            o_sb[:, n * cap:(n + 1) * cap],
                            psums[bank0 + n - n0][:, :cap],
                        )
                    # per-phase output DMA shortens the kernel tail; the last
                    # slots' outs ride HWDGE (lower completion latency than
                    # SWDGE on the critical exit path).  Outs must stay off
                    # the weight rings: an out dma_start waits on vector
                    # copies, and on a weight ring it would head-of-line
                    # block later slots' weight chunk issues.
                    oeng = nc.sync if s >= len(caps) - 2 else nc.gpsimd
                    oeng.dma_start(
                        out[:, NN * off + n0 * cap:NN * off + n1 * cap],
                        o_sb[:, n0 * cap:n1 * cap],
                    )
    nc.compile()
    return nc


def _run(inputs, trace=False):
    x = np.asarray(inputs["input"], dtype=np.float32)
    w = np.asarray(inputs["weight"], dtype=np.float32)
    counts = np.asarray(inputs["tokens_per_expert"], dtype=np.int64)
    starts = np.concatenate([[0], np.cumsum(counts)[:-1]])

    order = np.argsort(-counts, kind="stable")  # experts by size rank
    perm = list(range(EPC))  # largest-first; deep prefetch covers the tail
    caps = tuple(
        int(np.ceil(max(1, counts[order[r * NCORES:(r + 1) * NCORES]].max()) / 4) * 4)
        for r in perm
    )
    offs = np.concatenate([[0], np.cumsum(caps)]).astype(int)
    S = int(offs[-1])

    if caps not in _cache:
        _cache[caps] = _build(caps)
    nc = _cache[caps]

    # fp8 scale: w*s must fit in e3m4 (max normal 15.5); fold 1/s into x
    s_pow = 2.0 ** np.floor(np.log2(15.49 / np.abs(w).max()))
    x_sc = (x * (1.0 / s_pow)).astype(ml_dtypes.bfloat16)
    w8_full = (w * s_pow).astype(ml_dtypes.float8_e3m4)

    in_maps = []
    for c in range(NCORES):
        xt_pack = np.zeros((128, KC * S), dtype=ml_dtypes.bfloat16)
        w_pack = np.empty((EPC, 128, KC * DOUT), dtype=ml_dtypes.float8_e3m4)
        for s in range(EPC):
            g = int(order[perm[s] * NCORES + c])
            cnt = int(counts[g])
            cap = caps[s]
            o0 = KC * int(offs[s])
            if cnt:
                blk = np.zeros((128, KC, cap), dtype=ml_dtypes.bfloat16)
                blk[:, :, :cnt] = (
                    x_sc[starts[g]:starts[g] + cnt].T
                    .reshape(KC, 128, cnt).transpose(1, 0, 2)
                )
                xt_pack[:, o0:o0 + KC * cap] = blk.reshape(128, KC * cap)
            w_pack[s] = (
                w8_full[g].reshape(KC, 128, DOUT).transpose(1, 0, 2)
                .reshape(128, KC * DOUT)
            )
        in_maps.append({"w8": w_pack, "xt": xt_pack})

    kw = {"trace_cores": list(range(NCORES))} if trace else {}
    res = run_bass_kernel_spmd(nc, in_maps, core_ids=list(range(NCORES)),
                               trace=trace, **kw)

    out = np.empty((T, DOUT), dtype=np.float32)
    for c in range(NCORES):
        ob = res.results[c]["out"]
        for s in range(EPC):
            g = int(order[perm[s] * NCORES + c])
            cnt = int(counts[g])
            cap = caps[s]
            if cnt:
                blk = ob[:, NN * offs[s]:NN * offs[s] + NN * cap]
                blk = blk.reshape(128, NN, cap).transpose(2, 1, 0)
                out[starts[g]:starts[g] + cnt] = (
                    blk.reshape(cap, DOUT)[:cnt].astype(np.float32)
                )
    return out, res


def kernel(**inputs) -> np.ndarray:
    return _run(inputs)[0]


# revision 21
# speedup vs baseline: 1.0131x; 1.0131x over previous
"""Grouped GEMM (MoE routing) Trainium2 kernel — token-streaming fp8 design.

Expert-parallel across 8 NeuronCores with size-sorted slot assignment
(slot s on core c holds the expert of size-rank 8s+c; per-slot capacity
cap_s = roundup8(max count in rank group)).

Key design vs the bf16 token-stationary baseline:
- Weights are quantized to float8e3 (E3M4, 4 mantissa bits) on host with
  a global power-of-2 scale folded into x (y = (x/s) @ (w*s) exactly), so
  weight HBM traffic halves: 68 MB -> 34 MB per core.  Measured accuracy
  vs the fp32 reference: ~1.2e-2 max-rel (w-only quantization error; x
  stays bf16 as the PE moving operand, mixed-dtype matmul).
- Token-streaming orientation: stationary = w tile [128k x 128n] fp8,
  moving = x^T [128k x cap] bf16, psum [128n x cap].  PE cost scales with
  actual token count instead of ceil(count/128)*128, removing the ~1.5x
  m-tile padding of the token-stationary layout.
- Output written bf16 (adds ~0.2% error, halves write traffic), host
  upcasts to fp32.

Per-core HBM traffic: 34.1 w + ~5.9 xt + ~3.9 out = 44 MB (~125 us DMA
floor at the measured ~330-350 GB/s aggregate); PE ~135 us (stream-bound,
LDWEIGHTS fully hidden); measured 174 us vs the 276 us bf16 baseline.
"""
import ml_dtypes
import numpy as np

import concourse.mybir as mybir
import concourse.tile as tile
from concourse import bacc
from concourse.bass_utils import run_bass_kernel_spmd

G, T, DIN, DOUT = 64, 8192, 2560, 1664
NCORES = 8
EPC = G // NCORES   # expert slots per core
KC = DIN // 128     # 20 contraction chunks
NN = DOUT // 128    # 13 output-row chunks

_cache = {}


def _build(caps):
    caps = [int(c) for c in caps if c > 0]
    offs = np.concatenate([[0], np.cumsum(caps)]).astype(int)
    S = int(offs[-1])
    nc = bacc.Bacc(trn_type="TRN2", debug=False)
    f8 = mybir.dt.float8e3
    bf16 = mybir.dt.bfloat16
    f32 = mybir.dt.float32

    # partition-major layouts: every DMA below is a [128, N] slice whose
    # per-partition bytes are contiguous in HBM (large descriptors)
    w8 = nc.dram_tensor("w8", [EPC, 128, KC * DOUT], f8,
                        kind="ExternalInput").ap()
    xt = nc.dram_tensor("xt", [128, KC * S], bf16, kind="ExternalInput").ap()
    out = nc.dram_tensor("out", [128, NN * S], bf16, kind="ExternalOutput").ap()

    WB = 4   # k-chunks per w DMA
    NWCH = KC // WB  # 5 w chunks per slot
    with tile.TileContext(nc) as tc:
        with (
            tc.tile_pool(name="wp", bufs=3) as w_pool,
            tc.tile_pool(name="xp", bufs=1) as x_pool,
            tc.tile_pool(name="op", bufs=2) as o_pool,
            tc.tile_pool(name="ps", bufs=1, space="PSUM") as ps_pool,
        ):
            # PE warm-up: ~4us of dummy matmuls on a zeroed tile so the HAM
            # clock gate reaches 2.4 GHz before the first real matmul, and the
            # PE is busy while the first DMAs land.
            warm_l = x_pool.tile([128, 128], bf16, tag="wl", name="warm_l")
            warm_r = x_pool.tile([128, 512], bf16, tag="wr", name="warm_r")
            nc.vector.memset(warm_l[:], 0)
            nc.vector.memset(warm_r[:], 0)
            pswarm = ps_pool.tile([128, 512], f32, tag="psw", name="pswarm")
            for i in range(36):
                nc.tensor.matmul(pswarm[:], warm_l[:], warm_r[:],
                                 start=True, stop=True)
            psums = {}
            for j in range(7):  # one open accumulation region per bank
                psums[j] = ps_pool.tile([128, 512], f32, tag=f"ps{j}",
                                        name=f"psum{j}")
            psums[7] = pswarm  # warmup bank doubles as the 8th region

            for s, cap in enumerate(caps):
                off = int(offs[s])
                # per-slot x^T tile (1.3MB at cap 256): prefetched like the
                # weights; slots 0/1 ride the fast HWDGE rings so slot 0
                # starts right as the warmup ends, later slots go SWDGE
                xs = x_pool.tile([128, KC * cap], bf16, tag="xs",
                                 name=f"xs{s}", bufs=3)
                if s == 0:
                    # slot 0 is the startup critical path: split x^T across
                    # the two queues that don't carry w0 chunk 0
                    h = (KC // 2) * cap
                    nc.scalar.dma_start(xs[:, :h], xt[:, KC * off:KC * off + h])
                    nc.gpsimd.dma_start(
                        xs[:, h:], xt[:, KC * off + h:KC * (off + cap)]
                    )
                else:
                    xeng = nc.scalar if s == 1 else nc.gpsimd
                    xeng.dma_start(xs[:], xt[:, KC * off:KC * (off + cap)])
                # 852KB weight chunks; queue choreography: slot 0's chunks
                # sequenced so each arrives just before phase A consumes it,
                # tail slots use all three queues (the HWDGE rings alone
                # can't stream the tail while slots shorten)
                WQ = {
                    0: (nc.sync, nc.sync, nc.scalar, nc.sync, nc.scalar),
                    3: (nc.sync, nc.scalar, nc.sync, nc.scalar, nc.gpsimd),
                    4: (nc.sync, nc.scalar, nc.sync, nc.scalar, nc.gpsimd),
                    5: (nc.sync, nc.scalar, nc.sync, nc.scalar, nc.gpsimd),
                    6: (nc.scalar, nc.sync, nc.gpsimd, nc.scalar, nc.sync),
                    7: (nc.sync, nc.scalar, nc.gpsimd, nc.sync, nc.scalar),
                }
                wch = []
                for j in range(NWCH):
                    # tags 0/1 ride the HWDGE rings for every slot, where ring
                    # FIFO already orders them behind earlier slots' chunks —
                    # a deeper window there releases tail weights a slot
                    # earlier without letting gpsimd-routed future chunks
                    # steal SDMA share from the startup window
                    wj = w_pool.tile([128, WB * DOUT], f8, tag=f"w{j}",
                                     name=f"w{s}_{j}", bufs=4 if j < 2 else 3)
                    if s in WQ:
                        eng = WQ[s][j]
                    else:
                        eng = nc.sync if (s * NWCH + j) % 2 == 0 else nc.scalar
                    eng.dma_start(
                        wj[:], w8[s, :, j * WB * DOUT:(j + 1) * WB * DOUT]
                    )
                    wch.append(wj)
                o_sb = o_pool.tile([128, NN * cap], bf16, tag="o", name=f"o{s}")
                assert cap <= 256
                # k-outer within each n-phase: weight/xt chunks are consumed
                # progressively; each PSUM bank hosts exactly one open
                # accumulation region at a time (start/stop clear per bank)
                for n0, n1 in ((0, 8), (8, NN)):
                    for k in range(KC):
                        wk = wch[k // WB]
                        kb = (k % WB) * DOUT
                        for n in range(n0, n1):
                            ps = psums[n - n0][:, :cap]
                            nc.tensor.matmul(
                                ps,
                                wk[:, kb + n * 128:kb + (n + 1) * 128],
                                xs[:, k * cap:(k + 1) * cap],
                                start=(k == 0),
                                stop=(k == KC - 1),
                            )
                    for n in range(n0, n1):
                        nc.vector.tensor_copy(
                            o_sb[:, n * cap:(n + 1) * cap],
                            psums[n - n0][:, :cap],
                        )
                    # per-phase output DMA shortens the kernel tail; the last
                    # slots' outs ride HWDGE (lower completion latency than
                    # SWDGE on the critical exit path)
                    oeng = nc.sync if s >= len(caps) - 2 else nc.gpsimd
                    oeng.dma_start(
                        out[:, NN * off + n0 * cap:NN * off + n1 * cap],
                        o_sb[:, n0 * cap:n1 * cap],
                    )
    nc.compile()
    return nc


def _run(inputs, trace=False):
    x = np.asarray(inputs["input"], dtype=np.float32)
    w = np.asarray(inputs["weight"], dtype=np.float32)
    counts = np.asarray(inputs["tokens_per_expert"], dtype=np.int64)
    starts = np.concatenate([[0], np.cumsum(counts)[:-1]])

    order = np.argsort(-counts, kind="stable")  # experts by size rank
    perm = list(range(EPC))  # largest-first; deep prefetch covers the tail
    caps = tuple(
        int(np.ceil(max(1, counts[order[r * NCORES:(r + 1) * NCORES]].max()) / 4) * 4)
        for r in perm
    )
    offs = np.concatenate([[0], np.cumsum(caps)]).astype(int)
    S = int(offs[-1])

    if caps not in _cache:
        _cache[caps] = _build(caps)
    nc = _cache[caps]

    # fp8 scale: w*s must fit in e3m4 (max normal 15.5); fold 1/s into x
    s_pow = 2.0 ** np.floor(np.log2(15.49 / np.abs(w).max()))
    x_sc = (x * (1.0 / s_pow)).astype(ml_dtypes.bfloat16)
    w8_full = (w * s_pow).astype(ml_dtypes.float8_e3m4)

    in_maps = []
    for c in range(NCORES):
        xt_pack = np.zeros((128, KC * S), dtype=ml_dtypes.bfloat16)
        w_pack = np.empty((EPC, 128, KC * DOUT), dtype=ml_dtypes.float8_e3m4)
        for s in range(EPC):
            g = int(order[perm[s] * NCORES + c])
            cnt = int(counts[g])
            cap = caps[s]
            o0 = KC * int(offs[s])
            if cnt:
                blk = np.zeros((128, KC, cap), dtype=ml_dtypes.bfloat16)
                blk[:, :, :cnt] = (
                    x_sc[starts[g]:starts[g] + cnt].T
                    .reshape(KC, 128, cnt).transpose(1, 0, 2)
                )
                xt_pack[:, o0:o0 + KC * cap] = blk.reshape(128, KC * cap)
            w_pack[s] = (
                w8_full[g].reshape(KC, 128, DOUT).transpose(1, 0, 2)
                .reshape(128, KC * DOUT)
            )
        in_maps.append({"w8": w_pack, "xt": xt_pack})

    kw = {"trace_cores": list(range(NCORES))} if trace else {}
    res = run_bass_kernel_spmd(nc, in_maps, core_ids=list(range(NCORES)),
                               trace=trace, **kw)

    out = np.empty((T, DOUT), dtype=np.float32)
    for c in range(NCORES):
        ob = res.results[c]["out"]
        for s in range(EPC):
            g = int(order[perm[s] * NCORES + c])
            cnt = int(counts[g])
            cap = caps[s]
            if cnt:
                blk = ob[:, NN * offs[s]:NN * offs[s] + NN * cap]
                blk = blk.reshape(128, NN, cap).transpose(2, 1, 0)
                out[starts[g]:starts[g] + cnt] = (
                    blk.reshape(cap, DOUT)[:cnt].astype(np.float32)
                )
    return out, res


def kernel(**inputs) -> np.ndarray:
    return _run(inputs)[0]

